# Initial kernel scaffold
#
"""GAT (graph attention) Trainium2 kernel.

Full-input contract: kernel(**inputs) takes the unsharded tensors
  x   (8, 1024, 512) f32
  adj (8, 1024, 1024) i32
  W   (8, 256, 512) f32
  a1  (8, 256) f32
  a2  (8, 256) f32
and returns out (8, 1024, 256) f32.

Sharding: data-parallel over batch B=8 across the 8 NeuronCores; each core
computes all heads for one batch element. No collectives needed.

Per-core algorithm (N=1024 nodes, F_in=512, F_out=256, H=8 heads):
  h_h   = x @ W_h^T                       (bf16 matmul, fp32 accum)
  f1/f2 = x @ (W_h^T a)                   (fp32 matmul)
  e^T[j,i] = f1[i] + f2[j]                (K=1 PE broadcast + ACT bias)
  exp(lrelu(e)) = max(exp(e), exp(0.2 e)) (two ACT Exp passes)
  att^T = that * adj^T                    (DVE, bf16)
  o = att @ [h | 1]                       (bf16 matmul -> numerator + denom)
  elu(o/d) + 1 = min(exp(o/d),1) + max(o/d,0)   (+1 cancels in log_softmax)
  out = log_softmax(sum_h elu_h)
"""
import sys

sys.path.insert(0, "/opt/trn_rl_repo")

from contextlib import ExitStack

import numpy as np

import concourse.bacc as bacc
import concourse.bass as bass
import concourse.mybir as mybir
import concourse.tile as tile
from concourse import masks
from concourse._compat import with_exitstack

F32 = mybir.dt.float32
BF16 = mybir.dt.bfloat16
I32 = mybir.dt.int32
AF = mybir.ActivationFunctionType
ALU = mybir.AluOpType

N, F_IN, F_OUT, H, B = 1024, 512, 256, 8, 8
P = 128
NT = N // P        # 8 node tiles
FT = F_IN // P     # 4 f_in tiles
OT = F_OUT // P    # 2 f_out tiles
HB = F_OUT + 2     # per-head block in h_ext: 256 values + ones col + pad


@with_exitstack
def gat_kernel(ctx: ExitStack, tc, out_d, x_d, adj_d, W_d, a1_d, a2_d):
    nc = tc.nc

    const = ctx.enter_context(tc.tile_pool(name="const", bufs=1))
    ident = const.tile([P, P], F32, tag="ident")
    masks.make_identity(nc, ident[:])
    ones_row = const.tile([1, P], F32, tag="ones_row")
    nc.vector.memset(ones_row[:], 1.0)

    persist = ctx.enter_context(tc.tile_pool(name="persist", bufs=1))
    xT_bf = [persist.tile([P, N], BF16, tag=f"xTbf{fc}") for fc in range(FT)]
    WT_bf = [persist.tile([P, H * F_OUT], BF16, tag=f"WTbf{fc}") for fc in range(FT)]
    h_ext = [persist.tile([P, H * HB], BF16, tag=f"hext{nt}") for nt in range(NT)]
    adjT = [persist.tile([P, N], BF16, tag=f"adjT{jt}") for jt in range(NT)]
    f12 = [persist.tile([P, 16], F32, tag=f"f12_{nt}") for nt in range(NT)]
    f12s = [persist.tile([P, 16], F32, tag=f"f12s_{nt}") for nt in range(NT)]
    f1row = persist.tile([16, N], F32, tag="f1row")
    s_acc = [persist.tile([P, F_OUT], F32, tag=f"sacc{it}") for it in range(NT)]

    # ---------------- Stage A: loads, transposes, f1/f2 ----------------
    with ExitStack() as sa:
        pa = sa.enter_context(tc.tile_pool(name="stageA", bufs=3))
        pa2 = sa.enter_context(tc.tile_pool(name="stageA2", bufs=2))
        xtf_pool = sa.enter_context(tc.tile_pool(name="xtf", bufs=1))
        ps_a = sa.enter_context(tc.tile_pool(name="psA", bufs=4, space="PSUM"))

        xT_f32 = [xtf_pool.tile([P, N], F32, tag=f"xTf32{fc}") for fc in range(FT)]
        w12_sb = xtf_pool.tile([P, 64], F32, tag="w12")
        a12_sb = xtf_pool.tile([16, F_OUT], F32, tag="a12")

        # a1/a2 -> (16, 256) rows 0..7 = a1 heads, 8..15 = a2 heads
        nc.sync.dma_start(a12_sb[0:8, :], a1_d[:, :])
        nc.sync.dma_start(a12_sb[8:16, :], a2_d[:, :])

        # x transpose: x (n,f) -> xT (f,n), keep f32 + bf16 copies
        for nt in range(NT):
            xnat = pa.tile([P, F_IN], F32, tag="xnat")
            nc.sync.dma_start(xnat[:], x_d[nt * P:(nt + 1) * P, :])
            for fc in range(FT):
                pt = ps_a.tile([P, P], F32, tag="psA")
                nc.tensor.matmul(pt[:], xnat[:, fc * P:(fc + 1) * P], ident[:],
                                 is_transpose=True)
                nc.scalar.copy(xT_f32[fc][:, nt * P:(nt + 1) * P], pt[:])
        for fc in range(FT):
            nc.vector.tensor_copy(xT_bf[fc][:], xT_f32[fc][:])

        # a12 transpose: (16, 256) -> per ot (128, 16) on partitions
        a12T = xtf_pool.tile([P, 32], F32, tag="a12T")  # [p, ot*16 + (c h)]
        for ot in range(OT):
            pt = ps_a.tile([P, 16], F32, tag="psA_a")
            nc.tensor.matmul(pt[:], a12_sb[:, ot * P:(ot + 1) * P],
                             ident[0:16, 0:16], is_transpose=True)
            nc.scalar.copy(a12T[:, ot * 16:(ot + 1) * 16], pt[:])
        a12Tv = a12T[:].rearrange("p (t c h) -> p t c h", t=2, c=2)

        # W: load natural, transpose to WT_bf; w12 = W^T @ [a1 a2] (fp32)
        w12v = w12_sb[:].rearrange("p (fc c h) -> p fc c h", fc=FT, c=2)
        for h in range(H):
            wp = ps_a.tile([P, 8], F32, tag="psA_w")
            wnats = []
            for ot in range(OT):
                wnat = pa2.tile([P, F_IN], F32, tag="wnat")
                wnats.append(wnat)
                nc.sync.dma_start(wnat[:], W_d[h, ot * P:(ot + 1) * P, :])
                for fc in range(FT):
                    pt = ps_a.tile([P, P], F32, tag="psA")
                    nc.tensor.matmul(pt[:], wnat[:, fc * P:(fc + 1) * P],
                                     ident[:], is_transpose=True)
                    nc.scalar.copy(
                        WT_bf[fc][:, h * F_OUT + ot * P: h * F_OUT + (ot + 1) * P],
                        pt[:])
            for fc in range(FT):
                for ot in range(OT):
                    nc.tensor.matmul(
                        wp[:, fc * 2:(fc + 1) * 2],
                        wnats[ot][:, fc * P:(fc + 1) * P],
                        a12Tv[:, ot, :, h],
                        start=(ot == 0), stop=(ot == OT - 1))
            nc.scalar.copy(w12v[:, :, :, h], wp[:].rearrange("p (fc c) -> p fc c", fc=FT))

        # f1/f2 = x @ w12 (fp32): f12[nt] cols = c*8 + h
        for nt in range(NT):
            fp = ps_a.tile([P, 16], F32, tag="psA_f")
            for fc in range(FT):
                nc.tensor.matmul(fp[:], xT_f32[fc][:, nt * P:(nt + 1) * P],
                                 w12v[:, fc], start=(fc == 0), stop=(fc == FT - 1))
            nc.scalar.copy(f12[nt][:], fp[:])
            nc.vector.tensor_scalar_mul(f12s[nt][:], f12[nt][:], 0.2)
            # f1row rows: row (c*8+h); transpose (128,16) -> (16,128)
            ft = ps_a.tile([16, P], F32, tag="psA_fr")
            nc.tensor.matmul(ft[:], f12[nt][:], ident[:], is_transpose=True)
            nc.scalar.copy(f1row[:, nt * P:(nt + 1) * P], ft[:])

    # ---------------- Stage A2: adjacency cast + transpose ----------------
    with ExitStack() as sb:
        pj = sb.enter_context(tc.tile_pool(name="adjload", bufs=2))
        pjb = sb.enter_context(tc.tile_pool(name="adjcast", bufs=2))
        for it in range(NT):
            ai = pj.tile([P, N], I32, tag="adji")
            nc.sync.dma_start(ai[:], adj_d[it * P:(it + 1) * P, :])
            ab = pjb.tile([P, N], BF16, tag="adjb")
            nc.vector.tensor_scalar(ab[:], ai[:], 0, None, op0=ALU.add)
            for jt in range(NT):
                nc.sync.dma_start_transpose(
                    adjT[jt][:, it * P:(it + 1) * P],
                    ab[:, jt * P:(jt + 1) * P])

    # ---------------- Stage B: h = x @ W^T (bf16), build h_ext ----------------
    ps_h = ctx.enter_context(tc.tile_pool(name="psH", bufs=2, space="PSUM"))
    for nt in range(NT):
        hv = h_ext[nt][:].rearrange("p (h c) -> p h c", h=H)
        nc.vector.memset(hv[:, :, F_OUT:F_OUT + 1], 1.0)
        for hp in range(H // 2):  # head pairs -> N=512 matmuls
            hps = ps_h.tile([P, 2 * F_OUT], F32, tag="hpsum")
            for fc in range(FT):
                nc.tensor.matmul(hps[:], xT_bf[fc][:, nt * P:(nt + 1) * P],
                                 WT_bf[fc][:, hp * 2 * F_OUT:(hp + 1) * 2 * F_OUT],
                                 start=(fc == 0), stop=(fc == FT - 1))
            nc.vector.tensor_copy(h_ext[nt][:, (2 * hp) * HB:(2 * hp) * HB + F_OUT],
                                  hps[:, 0:F_OUT])
            nc.vector.tensor_copy(h_ext[nt][:, (2 * hp + 1) * HB:(2 * hp + 1) * HB + F_OUT],
                                  hps[:, F_OUT:2 * F_OUT])

    # ---------------- Stage C: per-head attention ----------------
    ps_e = ctx.enter_context(tc.tile_pool(name="psE", bufs=1, space="PSUM"))
    ps_o = ctx.enter_context(tc.tile_pool(name="psO", bufs=3, space="PSUM"))
    zp = ctx.enter_context(tc.tile_pool(name="zp", bufs=2))
    tmp_p = ctx.enter_context(tc.tile_pool(name="tmp", bufs=2))
    att_p = ctx.enter_context(tc.tile_pool(name="attp", bufs=12))
    ep = ctx.enter_context(tc.tile_pool(name="epilogue", bufs=3))

    for h in range(H):
        # f1 broadcast into PSUM: eb[j, i] = f1[i]
        eb = ps_e.tile([P, N], F32, tag="eb")
        for c in range(2):
            nc.tensor.matmul(eb[:, c * 512:(c + 1) * 512], ones_row[:],
                             f1row[h:h + 1, c * 512:(c + 1) * 512])
        atts = []
        for jt in range(NT):
            z1 = zp.tile([P, N], F32, tag="z1")
            nc.scalar.activation(z1[:], eb[:], AF.Exp,
                                 bias=f12[jt][:, 8 + h:9 + h], scale=1.0)
            z2 = zp.tile([P, N], F32, tag="z2")
            nc.scalar.activation(z2[:], eb[:], AF.Exp,
                                 bias=f12s[jt][:, 8 + h:9 + h], scale=0.2)
            tm = tmp_p.tile([P, N], BF16, tag="tm")
            nc.vector.tensor_max(tm[:], z1[:], z2[:])
            att = att_p.tile([P, N], BF16, tag="att")
            nc.vector.tensor_mul(att[:], tm[:], adjT[jt][:])
            atts.append(att)
        for it in range(NT):
            op = ps_o.tile([P, F_OUT + 1], F32, tag="opsum")
            for jt in range(NT):
                nc.tensor.matmul(op[:], atts[jt][:, it * P:(it + 1) * P],
                                 h_ext[jt][:, h * HB:h * HB + F_OUT + 1],
                                 start=(jt == 0), stop=(jt == NT - 1))
            rec = ep.tile([P, 1], F32, tag="rec")
            nc.vector.reciprocal(rec[:], op[:, F_OUT:F_OUT + 1])
            zt = ep.tile([P, F_OUT], F32, tag="zt")
            nc.scalar.activation(zt[:], op[:, 0:F_OUT], AF.Exp, scale=rec[:, 0:1])
            rt = ep.tile([P, F_OUT], F32, tag="rt")
            nc.vector.tensor_scalar(rt[:], op[:, 0:F_OUT], rec[:, 0:1], 0.0,
                                    op0=ALU.mult, op1=ALU.max)
            if h == 0:
                nc.vector.scalar_tensor_tensor(s_acc[it][:], zt[:], 1.0, rt[:],
                                               op0=ALU.min, op1=ALU.add)
            else:
                ut = ep.tile([P, F_OUT], F32, tag="ut")
                nc.vector.scalar_tensor_tensor(ut[:], zt[:], 1.0, rt[:],
                                               op0=ALU.min, op1=ALU.add)
                nc.vector.tensor_add(s_acc[it][:], s_acc[it][:], ut[:])

    # ---------------- Stage D: log_softmax over F_OUT ----------------
    dp = ctx.enter_context(tc.tile_pool(name="lsm", bufs=2))
    for it in range(NT):
        zz = dp.tile([P, F_OUT], F32, tag="zz")
        ds = dp.tile([P, 1], F32, tag="ds")
        nc.scalar.activation(zz[:], s_acc[it][:], AF.Exp, accum_out=ds[:, 0:1])
        lnd = dp.tile([P, 1], F32, tag="lnd")
        nc.scalar.activation(lnd[:], ds[:], AF.Ln)
        ot_t = dp.tile([P, F_OUT], F32, tag="outt")
        nc.vector.tensor_scalar(ot_t[:], s_acc[it][:], lnd[:, 0:1], None,
                                op0=ALU.subtract)
        nc.sync.dma_start(out_d[it * P:(it + 1) * P, :], ot_t[:])


_PROGRAM_CACHE = {}


def build_gat_program():
    if "nc" in _PROGRAM_CACHE:
        return _PROGRAM_CACHE["nc"]
    nc = bacc.Bacc("TRN2", debug=False)
    x_d = nc.dram_tensor("x", (N, F_IN), F32, kind="ExternalInput").ap()
    adj_d = nc.dram_tensor("adj", (N, N), I32, kind="ExternalInput").ap()
    W_d = nc.dram_tensor("W", (H, F_OUT, F_IN), F32, kind="ExternalInput").ap()
    a1_d = nc.dram_tensor("a1", (H, F_OUT), F32, kind="ExternalInput").ap()
    a2_d = nc.dram_tensor("a2", (H, F_OUT), F32, kind="ExternalInput").ap()
    out_d = nc.dram_tensor("out", (N, F_OUT), F32, kind="ExternalOutput").ap()
    with tile.TileContext(nc) as tc:
        gat_kernel(tc, out_d, x_d, adj_d, W_d, a1_d, a2_d)
    nc.compile()
    _PROGRAM_CACHE["nc"] = nc
    return nc


def kernel(x, adj, W, a1, a2, _trace=False):
    from concourse.bass_utils import run_bass_kernel_spmd

    x = np.ascontiguousarray(np.asarray(x, dtype=np.float32))
    adj = np.ascontiguousarray(np.asarray(adj, dtype=np.int32))
    W = np.ascontiguousarray(np.asarray(W, dtype=np.float32))
    a1 = np.ascontiguousarray(np.asarray(a1, dtype=np.float32))
    a2 = np.ascontiguousarray(np.asarray(a2, dtype=np.float32))

    nc = build_gat_program()
    in_maps = [{"x": x[b], "adj": adj[b], "W": W, "a1": a1, "a2": a2}
               for b in range(B)]
    res = run_bass_kernel_spmd(nc, in_maps, core_ids=list(range(B)),
                               trace=_trace)
    out = np.stack([res.results[b]["out"] for b in range(B)])
    if _trace:
        kernel.last_result = res
    return out


# revision 10
# speedup vs baseline: 1.2276x; 1.2276x over previous
"""GAT (graph attention) Trainium2 kernel.

Full-input contract: kernel(**inputs) takes the unsharded tensors
  x   (8, 1024, 512) f32
  adj (8, 1024, 1024) i32
  W   (8, 256, 512) f32
  a1  (8, 256) f32
  a2  (8, 256) f32
and returns out (8, 1024, 256) f32.

Sharding: data-parallel over batch B=8 across the 8 NeuronCores; each core
computes all heads for one batch element. No collectives needed.

Per-core algorithm (N=1024 nodes, F_in=512, F_out=256, H=8 heads):
  h_h   = x @ W_h^T                       (bf16 matmul, fp32 accum)
  f1/f2 = x @ (W_h^T a)                   (fp32 matmul)
  e^T[j,i] = f1[i] + f2[j]                (K=1 PE broadcast + ACT bias)
  exp(lrelu(e)) = max(exp(e), exp(0.2 e)) (two ACT Exp passes)
  att^T = that * adj^T                    (DVE, bf16)
  o = att @ [h | 1]                       (bf16 matmul -> numerator + denom)
  elu(o/d) + 1 = min(exp(o/d),1) + max(o/d,0)   (+1 cancels in log_softmax)
  out = log_softmax(sum_h elu_h)
"""
import sys

sys.path.insert(0, "/opt/trn_rl_repo")

from contextlib import ExitStack

import numpy as np

import concourse.bacc as bacc
import concourse.bass as bass
import concourse.mybir as mybir
import concourse.tile as tile
from concourse import masks
from concourse._compat import with_exitstack

F32 = mybir.dt.float32
BF16 = mybir.dt.bfloat16
I32 = mybir.dt.int32
AF = mybir.ActivationFunctionType
ALU = mybir.AluOpType

N, F_IN, F_OUT, H, B = 1024, 512, 256, 8, 8
P = 128
NT = N // P        # 8 node tiles
FT = F_IN // P     # 4 f_in tiles
OT = F_OUT // P    # 2 f_out tiles
HB = F_OUT + 2     # per-head block in h_ext: 256 values + ones col + pad


@with_exitstack
def gat_kernel(ctx: ExitStack, tc, out_d, x_d, adj_d, W_d, a1_d, a2_d):
    nc = tc.nc

    const = ctx.enter_context(tc.tile_pool(name="const", bufs=1))
    ident = const.tile([P, P], F32, name="ident", tag="ident")
    masks.make_identity(nc, ident[:])
    ones_row = const.tile([1, P], F32, name="ones_row", tag="ones_row")
    nc.vector.memset(ones_row[:], 1.0)
    ident_bf = const.tile([P, P], BF16, name="ident_bf", tag="ident_bf")
    masks.make_identity(nc, ident_bf[:])

    persist = ctx.enter_context(tc.tile_pool(name="persist", bufs=1))
    xT_bf = [persist.tile([P, N], BF16, name=f"xTbf{fc}", tag=f"xTbf{fc}") for fc in range(FT)]
    WT_bf = [persist.tile([P, H * F_OUT], BF16, name=f"WTbf{fc}", tag=f"WTbf{fc}") for fc in range(FT)]
    h_ext = [persist.tile([P, H * HB], BF16, name=f"hext{nt}", tag=f"hext{nt}") for nt in range(NT)]
    adjT = [persist.tile([P, N], BF16, name=f"adjT{jt}", tag=f"adjT{jt}") for jt in range(NT)]
    f12 = [persist.tile([P, 16], F32, name=f"f12_{nt}", tag=f"f12_{nt}") for nt in range(NT)]
    f12s = [persist.tile([P, 16], F32, name=f"f12s_{nt}", tag=f"f12s_{nt}") for nt in range(NT)]
    # f1 per head as a partition-0 row (matmul rhs base partition must be 0)
    f1flat = persist.tile([1, H * N], F32, name="f1flat", tag="f1flat")
    s_acc = [persist.tile([P, F_OUT], F32, name=f"sacc{it}", tag=f"sacc{it}") for it in range(NT)]

    # ---------------- Stage A: loads, transposes, f1/f2 ----------------
    with ExitStack() as sa:
        pa = sa.enter_context(tc.tile_pool(name="stageA", bufs=3))
        pa2 = sa.enter_context(tc.tile_pool(name="stageA2", bufs=2))
        xtf_pool = sa.enter_context(tc.tile_pool(name="xtf", bufs=1))
        ps_a = sa.enter_context(tc.tile_pool(name="psA", bufs=3, space="PSUM"))
        ps_aa = sa.enter_context(tc.tile_pool(name="psAa", bufs=1, space="PSUM"))
        ps_aw = sa.enter_context(tc.tile_pool(name="psAw", bufs=2, space="PSUM"))
        ps_af = sa.enter_context(tc.tile_pool(name="psAf", bufs=2, space="PSUM"))

        xT_f32 = [xtf_pool.tile([P, N], F32, name=f"xTf32{fc}", tag=f"xTf32{fc}") for fc in range(FT)]
        w12_sb = xtf_pool.tile([P, 64], F32, name="w12", tag="w12")
        a12_sb = xtf_pool.tile([16, F_OUT], F32, name="a12", tag="a12")

        # a1/a2 -> (16, 256) rows 0..7 = a1 heads, 8..15 = a2 heads
        nc.sync.dma_start(a12_sb[0:8, :], a1_d[:, :])
        nc.sync.dma_start(a12_sb[8:16, :], a2_d[:, :])

        # x transpose: x (n,f) -> xT (f,n), keep f32 + bf16 copies
        for nt in range(NT):
            xnat = pa.tile([P, F_IN], F32, name="xnat", tag="xnat")
            nc.sync.dma_start(xnat[:], x_d[nt * P:(nt + 1) * P, :])
            for fc in range(FT):
                pt = ps_a.tile([P, P], F32, name="psA", tag="psA")
                nc.tensor.matmul(pt[:], xnat[:, fc * P:(fc + 1) * P], ident[:],
                                 is_transpose=True)
                nc.scalar.copy(xT_f32[fc][:, nt * P:(nt + 1) * P], pt[:])
        for fc in range(FT):
            nc.vector.tensor_copy(xT_bf[fc][:], xT_f32[fc][:])

        # a12 transpose: (16, 256) -> per ot (128, 16) on partitions
        a12T = xtf_pool.tile([P, 32], F32, name="a12T", tag="a12T")  # [p, ot*16 + (c h)]
        for ot in range(OT):
            pt = ps_aa.tile([P, 16], F32, name="psA_a", tag="psA_a")
            nc.tensor.matmul(pt[:], a12_sb[:, ot * P:(ot + 1) * P],
                             ident[0:16, 0:16], is_transpose=True)
            nc.scalar.copy(a12T[:, ot * 16:(ot + 1) * 16], pt[:])
        a12Tv = a12T[:].rearrange("p (t c h) -> p t c h", t=2, c=2)

        # W: load natural, transpose to WT_bf; w12 = W^T @ [a1 a2] (fp32)
        w12v = w12_sb[:].rearrange("p (fc c h) -> p fc c h", fc=FT, c=2)
        for h in range(H):
            wp = ps_aw.tile([P, 8], F32, name="psA_w", tag="psA_w")
            wnats = []
            for ot in range(OT):
                wnat = pa2.tile([P, F_IN], F32, name="wnat", tag="wnat")
                wnats.append(wnat)
                nc.sync.dma_start(wnat[:], W_d[h, ot * P:(ot + 1) * P, :])
                for fc in range(FT):
                    pt = ps_a.tile([P, P], F32, name="psA", tag="psA")
                    nc.tensor.matmul(pt[:], wnat[:, fc * P:(fc + 1) * P],
                                     ident[:], is_transpose=True)
                    nc.scalar.copy(
                        WT_bf[fc][:, h * F_OUT + ot * P: h * F_OUT + (ot + 1) * P],
                        pt[:])
            for fc in range(FT):
                for ot in range(OT):
                    nc.tensor.matmul(
                        wp[:, fc * 2:(fc + 1) * 2],
                        wnats[ot][:, fc * P:(fc + 1) * P],
                        a12Tv[:, ot, :, h],
                        start=(ot == 0), stop=(ot == OT - 1))
            nc.scalar.copy(w12v[:, :, :, h], wp[:].rearrange("p (fc c) -> p fc c", fc=FT))

        # f1/f2 = x @ w12 (fp32): f12[nt] cols = c*8 + h
        for nt in range(NT):
            fp = ps_af.tile([P, 16], F32, name="psA_f", tag="psA_f")
            for fc in range(FT):
                nc.tensor.matmul(fp[:], xT_f32[fc][:, nt * P:(nt + 1) * P],
                                 w12v[:, fc], start=(fc == 0), stop=(fc == FT - 1))
            nc.scalar.copy(f12[nt][:], fp[:])
            nc.vector.tensor_scalar_mul(f12s[nt][:], f12[nt][:], 0.2)
            # gather f1 (col h of f12) into the head's partition-0 row
            for h in range(H):
                nc.sync.dma_start(f1flat[0:1, h * N + nt * P: h * N + (nt + 1) * P],
                                  f12[nt][:, h:h + 1])

    # ---------------- Stage A2: adjacency cast + transpose ----------------
    with ExitStack() as sb:
        pj = sb.enter_context(tc.tile_pool(name="adjload", bufs=2))
        pjb = sb.enter_context(tc.tile_pool(name="adjcast", bufs=2))
        ps_t = sb.enter_context(tc.tile_pool(name="psT", bufs=3, space="PSUM"))
        for it in range(NT):
            ai = pj.tile([P, N], I32, name="adji", tag="adji")
            nc.sync.dma_start(ai[:], adj_d[it * P:(it + 1) * P, :])
            ab = pjb.tile([P, N], BF16, name="adjb", tag="adjb")
            nc.vector.tensor_scalar(ab[:], ai[:], 0, None, op0=ALU.add)
            for jt in range(NT):
                pt = ps_t.tile([P, P], BF16, name="psT", tag="psT")
                nc.tensor.matmul(pt[:], ab[:, jt * P:(jt + 1) * P], ident_bf[:],
                                 is_transpose=True)
                nc.vector.tensor_copy(adjT[jt][:, it * P:(it + 1) * P], pt[:])

    # ---------------- Stage B: h = x @ W^T (bf16), build h_ext ----------------
    ps_h = ctx.enter_context(tc.tile_pool(name="psH", bufs=2, space="PSUM"))
    for nt in range(NT):
        hv = h_ext[nt][:].rearrange("p (h c) -> p h c", h=H)
        nc.vector.memset(hv[:, :, F_OUT:F_OUT + 1], 1.0)
        for hp in range(H // 2):  # head pairs -> N=512 matmuls
            hps = ps_h.tile([P, 2 * F_OUT], F32, name="hpsum", tag="hpsum")
            for fc in range(FT):
                nc.tensor.matmul(hps[:], xT_bf[fc][:, nt * P:(nt + 1) * P],
                                 WT_bf[fc][:, hp * 2 * F_OUT:(hp + 1) * 2 * F_OUT],
                                 start=(fc == 0), stop=(fc == FT - 1))
            nc.vector.tensor_copy(h_ext[nt][:, (2 * hp) * HB:(2 * hp) * HB + F_OUT],
                                  hps[:, 0:F_OUT])
            nc.vector.tensor_copy(h_ext[nt][:, (2 * hp + 1) * HB:(2 * hp + 1) * HB + F_OUT],
                                  hps[:, F_OUT:2 * F_OUT])

    # ---------------- Stage C: per-head attention ----------------
    ps_e = ctx.enter_context(tc.tile_pool(name="psE", bufs=1, space="PSUM"))
    ps_o = ctx.enter_context(tc.tile_pool(name="psO", bufs=3, space="PSUM"))
    zp = ctx.enter_context(tc.tile_pool(name="zp", bufs=2))
    tmp_p = ctx.enter_context(tc.tile_pool(name="tmp", bufs=2))
    att_p = ctx.enter_context(tc.tile_pool(name="attp", bufs=12))
    ep = ctx.enter_context(tc.tile_pool(name="epilogue", bufs=3))

    for h in range(H):
        # f1 broadcast into PSUM: eb[j, i] = f1[i]
        eb = ps_e.tile([P, N], F32, name="eb", tag="eb")
        for c in range(2):
            nc.tensor.matmul(eb[:, c * 512:(c + 1) * 512], ones_row[:],
                             f1flat[0:1, h * N + c * 512: h * N + (c + 1) * 512])
        atts = []
        for jt in range(NT):
            z1 = zp.tile([P, N], F32, name="z1", tag="z1")
            nc.scalar.activation(z1[:], eb[:], AF.Exp,
                                 bias=f12[jt][:, 8 + h:9 + h], scale=1.0)
            z2 = zp.tile([P, N], F32, name="z2", tag="z2")
            nc.scalar.activation(z2[:], eb[:], AF.Exp,
                                 bias=f12s[jt][:, 8 + h:9 + h], scale=0.2)
            tm = tmp_p.tile([P, N], BF16, name="tm", tag="tm")
            nc.vector.tensor_max(tm[:], z1[:], z2[:])
            att = att_p.tile([P, N], BF16, name="att", tag="att")
            nc.vector.tensor_mul(att[:], tm[:], adjT[jt][:])
            atts.append(att)
        for it in range(NT):
            op = ps_o.tile([P, F_OUT + 1], F32, name="opsum", tag="opsum")
            for jt in range(NT):
                nc.tensor.matmul(op[:], atts[jt][:, it * P:(it + 1) * P],
                                 h_ext[jt][:, h * HB:h * HB + F_OUT + 1],
                                 start=(jt == 0), stop=(jt == NT - 1))
            rec = ep.tile([P, 1], F32, name="rec", tag="rec")
            nc.vector.reciprocal(rec[:], op[:, F_OUT:F_OUT + 1])
            zt = ep.tile([P, F_OUT], F32, name="zt", tag="zt")
            nc.scalar.activation(zt[:], op[:, 0:F_OUT], AF.Exp, scale=rec[:, 0:1])
            rt = ep.tile([P, F_OUT], F32, name="rt", tag="rt")
            nc.vector.tensor_scalar(rt[:], op[:, 0:F_OUT], rec[:, 0:1], 0.0,
                                    op0=ALU.mult, op1=ALU.max)
            if h == 0:
                nc.vector.scalar_tensor_tensor(s_acc[it][:], zt[:], 1.0, rt[:],
                                               op0=ALU.min, op1=ALU.add)
            else:
                ut = ep.tile([P, F_OUT], F32, name="ut", tag="ut")
                nc.vector.scalar_tensor_tensor(ut[:], zt[:], 1.0, rt[:],
                                               op0=ALU.min, op1=ALU.add)
                nc.vector.tensor_add(s_acc[it][:], s_acc[it][:], ut[:])

    # ---------------- Stage D: log_softmax over F_OUT ----------------
    dp = ctx.enter_context(tc.tile_pool(name="lsm", bufs=2))
    for it in range(NT):
        zz = dp.tile([P, F_OUT], F32, name="zz", tag="zz")
        ds = dp.tile([P, 1], F32, name="ds", tag="ds")
        nc.scalar.activation(zz[:], s_acc[it][:], AF.Exp, accum_out=ds[:, 0:1])
        lnd = dp.tile([P, 1], F32, name="lnd", tag="lnd")
        nc.scalar.activation(lnd[:], ds[:], AF.Ln)
        ot_t = dp.tile([P, F_OUT], F32, name="outt", tag="outt")
        nc.vector.tensor_scalar(ot_t[:], s_acc[it][:], lnd[:, 0:1], None,
                                op0=ALU.subtract)
        nc.sync.dma_start(out_d[it * P:(it + 1) * P, :], ot_t[:])


_PROGRAM_CACHE = {}


def build_gat_program(repeats=1):
    key = ("nc", repeats)
    if key in _PROGRAM_CACHE:
        return _PROGRAM_CACHE[key]
    nc = bacc.Bacc("TRN2", debug=False)
    x_d = nc.dram_tensor("x", (N, F_IN), F32, kind="ExternalInput").ap()
    adj_d = nc.dram_tensor("adj", (N, N), I32, kind="ExternalInput").ap()
    W_d = nc.dram_tensor("W", (H, F_OUT, F_IN), F32, kind="ExternalInput").ap()
    a1_d = nc.dram_tensor("a1", (H, F_OUT), F32, kind="ExternalInput").ap()
    a2_d = nc.dram_tensor("a2", (H, F_OUT), F32, kind="ExternalInput").ap()
    out_d = nc.dram_tensor("out", (N, F_OUT), F32, kind="ExternalOutput").ap()
    with tile.TileContext(nc) as tc:
        for _ in range(repeats):
            gat_kernel(tc, out_d, x_d, adj_d, W_d, a1_d, a2_d)
    nc.compile()
    _PROGRAM_CACHE[key] = nc
    return nc


def kernel(x, adj, W, a1, a2, _trace=False):
    from concourse.bass_utils import run_bass_kernel_spmd

    x = np.ascontiguousarray(np.asarray(x, dtype=np.float32))
    adj = np.ascontiguousarray(np.asarray(adj, dtype=np.int32))
    W = np.ascontiguousarray(np.asarray(W, dtype=np.float32))
    a1 = np.ascontiguousarray(np.asarray(a1, dtype=np.float32))
    a2 = np.ascontiguousarray(np.asarray(a2, dtype=np.float32))

    nc = build_gat_program()
    in_maps = [{"x": x[b], "adj": adj[b], "W": W, "a1": a1, "a2": a2}
               for b in range(B)]
    res = run_bass_kernel_spmd(nc, in_maps, core_ids=list(range(B)),
                               trace=_trace)
    out = np.stack([res.results[b]["out"] for b in range(B)])
    if _trace:
        kernel.last_result = res
    return out
